# revision 30
# baseline (speedup 1.0000x reference)
"""MinGRU layer kernel for Trainium2 (8 NeuronCores, data-parallel over batch).

Math per batch element b (reference semantics):
    z_t = Wz @ x_t + bz ; g_t = sigmoid(z_t)
    u_t = Wh @ x_t + bh
    h_t = (1-g_t) * h_{t-1} + g_t * u_t     (linear recurrence along T)
    y_t = Wo @ h_t + bo

Device layout: hidden dim on partitions (8 tiles x 128), time on the free
dim, chunked by TC=512 columns. The recurrence runs on the DVE
``tensor_tensor_scan`` instruction (state = a*state + b along the free dim)
with a = sigmoid(-z-bz) = 1-g and b = (u+bh)*g. Matmuls take bf16 inputs
with fp32 PSUM accumulation; scan inputs stay fp32; h is stored bf16 for
the output matmul. Output-chunk matmuls are deferred one chunk so the PE
never waits on the serial scan chain.

DMA plan: the 16 shared DMA engines are line-count limited for small
lines, so x is stored chunk-major in DRAM ([P, NCH, KI, TC], 8 KB
contiguous per partition per chunk) and biases are packed into one
tensor. Startup runs the critical transfers on all three DGE queues in
parallel (sync: wz m0/m1 + biases; scalar: x chunk 0 in two half-K
blocks; gpsimd: wh m0/m1), and everything else (x chunk 1, wz/wh m2+,
wo) is gated behind the first matmul in consumption order.

Sharding: batch B=8 -> one batch element per core; weights broadcast.
"""

import numpy as np
import ml_dtypes

import concourse.bass as bass
import concourse.bacc as bacc
import concourse.mybir as mybir
import concourse.tile as tile
from concourse.bass_utils import run_bass_kernel_spmd
from concourse.bass_interp import get_hw_module
from concourse.tile_rust import add_dep_helper

B, T, I, H, O = 8, 4096, 1024, 1024, 1024
P = 128
TC = 512  # time chunk (matmul free dim / PSUM bank)

BF16 = mybir.dt.bfloat16
F32 = mybir.dt.float32
NPBF16 = ml_dtypes.bfloat16

AL = mybir.AluOpType
AF = mybir.ActivationFunctionType


def build_program(t=T, i=I, h=H, o=O, tc_len=TC, n_cores=8, enable_asserts=False):
    KI, MH, MO, NCH = i // P, h // P, o // P, t // tc_len
    nc = bacc.Bacc(
        "TRN2",
        target_bir_lowering=False,
        debug=False,
        enable_asserts=enable_asserts,
        num_devices=n_cores,
    )

    # Host pre-tiled layouts (see kernel() below for the exact packing).
    xT = nc.dram_tensor("xT", [P, NCH, KI, tc_len], BF16, kind="ExternalInput")
    wz = nc.dram_tensor("wz", [P, MH, KI, P], BF16, kind="ExternalInput")
    wh = nc.dram_tensor("wh", [P, MH, KI, P], BF16, kind="ExternalInput")
    wo = nc.dram_tensor("wo", [P, MO, MH, P], BF16, kind="ExternalInput")
    # bz | nbz | bh | bo side by side so one DMA moves all biases.
    biasd = nc.dram_tensor("biases", [P, 3 * MH + MO], F32, kind="ExternalInput")
    out = nc.dram_tensor("out", [P, MO, t], F32, kind="ExternalOutput")

    with tile.TileContext(nc, pool_alloc_mode="queue") as tcx:
        with (
            tcx.tile_pool(name="weights", bufs=1) as wpool,
            tcx.tile_pool(name="xab", bufs=2) as xpool,
            tcx.tile_pool(name="gtmp", bufs=4) as gpool,
            tcx.tile_pool(name="hsb", bufs=3) as hpool,
            tcx.tile_pool(name="psum", bufs=4, space=bass.MemorySpace.PSUM) as zups,
        ):
            # Tag-sharing pool aliases (bufs is per tag): x/a/b ride one
            # double-buffered pool; g/warm/osb share the 4-deep pool; the
            # zu and o PSUM rings are 4 banks each (8 total).
            abpool = xpool
            opool = gpool
            ops = zups
            x_first = xpool.tile([P, KI, tc_len], BF16, tag="x")
            wz_s = wpool.tile([P, MH, KI, P], BF16, tag="wz")
            wh_s = wpool.tile([P, MH, KI, P], BF16, tag="wh")
            wo_s = wpool.tile([P, MO, MH, P], BF16, tag="wo")
            bias_s = wpool.tile([P, 3 * MH + MO], F32, tag="biases")
            bz_s = bias_s[:, 0:MH]
            nbz_s = bias_s[:, MH : 2 * MH]
            bh_s = bias_s[:, 2 * MH : 3 * MH]
            bo_s = bias_s[:, 3 * MH : 3 * MH + MO]

            # Pre-warm the PE while the startup DMAs are in flight: the HAM
            # clock gate needs ~3us of continuous matmul work to reach full
            # speed, which these throwaway matmuls provide so the real
            # stream starts at 2.4 GHz. The scratch memset runs on the
            # vector engine (whose queue is otherwise empty at startup —
            # the DMA-issuing engines would delay it); the PSUM tile comes
            # from the normal zu ring and is recycled, never read.
            warm_sb = gpool.tile([P, tc_len], BF16, tag="warm")
            nc.vector.memset(warm_sb[:], 0.0)
            warm_ps = zups.tile([P, tc_len], F32, tag="zu")

            def emit_warm(n):
                for _ in range(n):
                    nc.tensor.matmul(
                        warm_ps[:], warm_sb[:, 0:P], warm_sb[:], start=True, stop=True
                    )

            # ~10 x 427ns at the throttled clock covers the ~3.4us of
            # continuous work the HAM clock gate needs plus the wait for
            # x k0-1, so the PE is at 2.4 GHz with no idle gap when the
            # first real matmul's data lands.
            emit_warm(10)

            # Critical startup transfers, three queues in parallel. The DMA
            # engines round-robin across queues per descriptor, so each
            # queue is ordered by when its data is consumed, and the
            # ungated preload is kept to ~1.5 MB (x chunk 0 + wz/wh m0 +
            # biases) so the matmul stream is continuous from ~10us on.
            # The m1+ weights join the gated stream below in consumption
            # order.
            half = KI // 2
            nc.scalar.dma_start(x_first[:, 0:2], xT[:, 0, 0:2])
            nc.sync.dma_start(wz_s[:, 0], wz[:, 0])
            nc.gpsimd.dma_start(x_first[:, half : half + 2], xT[:, 0, half : half + 2])
            nc.scalar.dma_start(x_first[:, 2:half], xT[:, 0, 2:half])
            nc.sync.dma_start(x_first[:, half + 2 : KI], xT[:, 0, half + 2 : KI])
            nc.gpsimd.dma_start(wh_s[:, 0], wh[:, 0])
            nc.sync.dma_start(bias_s[:], biasd[:])

            def emit_out_chunk(c, h_tile, final=False):
                sl = slice(c * tc_len, (c + 1) * tc_len)
                for mo in range(MO):
                    if final and mo == MO - 1:
                        # The very last output tile is the kernel's critical
                        # tail (bias-add -> store -> completion semaphore
                        # gates the NEFF drain). Accumulate it as two
                        # half-width PSUM groups with independent consumer
                        # chains on separate engines and DMA queues —
                        # PSUM-tile readers are serialized by the dep
                        # tracker, so a shared accumulator would run the
                        # halves back-to-back.
                        hl = tc_len // 2
                        sl_a = slice(c * tc_len, c * tc_len + hl)
                        sl_b = slice(c * tc_len + hl, (c + 1) * tc_len)
                        o_psa = ops.tile([P, tc_len], F32, tag="o")
                        o_psb = ops.tile([P, tc_len], F32, tag="o")
                        for k in range(MH):
                            nc.tensor.matmul(
                                o_psa[:, 0:hl],
                                wo_s[:, mo, k, :],
                                h_tile[:, k, 0:hl],
                                start=(k == 0),
                                stop=(k == MH - 1),
                            )
                            nc.tensor.matmul(
                                o_psb[:, 0:hl],
                                wo_s[:, mo, k, :],
                                h_tile[:, k, hl:tc_len],
                                start=(k == 0),
                                stop=(k == MH - 1),
                            )
                        o_sb = opool.tile([P, tc_len], F32, tag="osb")
                        nc.vector.tensor_scalar_add(
                            o_sb[:, hl:tc_len], o_psb[:, 0:hl],
                            bo_s[:, mo : mo + 1],
                        )
                        nc.scalar.activation(
                            o_sb[:, 0:hl],
                            o_psa[:, 0:hl],
                            AF.Identity,
                            bias=bo_s[:, mo : mo + 1],
                        )
                        nc.scalar.dma_start(out[:, mo, sl_b], o_sb[:, hl:tc_len])
                        nc.sync.dma_start(out[:, mo, sl_a], o_sb[:, 0:hl])
                        continue
                    o_ps = ops.tile([P, tc_len], F32, tag="o")
                    for k in range(MH):
                        nc.tensor.matmul(
                            o_ps[:],
                            wo_s[:, mo, k, :],
                            h_tile[:, k, :],
                            start=(k == 0),
                            stop=(k == MH - 1),
                        )
                    o_sb = opool.tile([P, tc_len], F32, tag="osb")
                    # Bias-add on the scalar engine: keeps o-PSUM recycling
                    # off the DVE queue (which carries the scan chain).
                    nc.scalar.activation(
                        o_sb[:], o_ps[:], AF.Identity, bias=bo_s[:, mo : mo + 1]
                    )
                    nc.sync.dma_start(out[:, mo, sl], o_sb[:])

            x_next = None  # chunk-1 x tile, prefetched from the gated block
            h_prev = None
            for c in range(NCH):
                if c == 0:
                    x_s = x_first
                elif x_next is not None:
                    x_s, x_next = x_next, None
                else:
                    x_s = xpool.tile([P, KI, tc_len], BF16, tag="x")
                    nc.gpsimd.dma_start(x_s[:], xT[:, c])

                a_s = abpool.tile([P, MH, tc_len], F32, tag="a")
                b_s = abpool.tile([P, MH, tc_len], F32, tag="b")
                h_s = hpool.tile([P, MH, tc_len], BF16, tag="h")

                for m in range(MH):
                    z_ps = zups.tile([P, tc_len], F32, tag="zu")
                    for k in range(KI):
                        if c == 0 and m == 0 and k == 2:
                            # x chunk 0's k2..7 blocks trail the first two
                            # k-slices by ~2.5us of DMA supply; keep the PE
                            # continuously busy through that window so the
                            # HAM clock never re-throttles.
                            emit_warm(11)
                        mm = nc.tensor.matmul(
                            z_ps[:],
                            wz_s[:, m, k, :],
                            x_s[:, k, :],
                            start=(k == 0),
                            stop=(k == KI - 1),
                        )
                        if c == 0 and m == 0 and k == 0 and MH > 1:
                            # wz/wh m1 are the next weights the PE needs
                            # after the m0 pair; release them as soon as
                            # the first matmul confirms the critical
                            # pieces landed. They ride sync (behind the x
                            # k6-7 block) and scalar (behind x k0-3), the
                            # two queues whose engines are free early —
                            # gpsimd's issue pipeline is congested until
                            # ~20us.
                            d = nc.sync.dma_start(wz_s[:, 1], wz[:, 1])
                            add_dep_helper(d.ins, mm.ins, True, "wz m1 after start")
                            d = nc.scalar.dma_start(wh_s[:, 1], wh[:, 1])
                            add_dep_helper(d.ins, mm.ins, True, "wh m1 after start")
                        if c == 0 and m == 0 and k == 4:
                            # Everything not needed for the first m group
                            # waits for mid-group-0, which lands after the
                            # x chunk 0 tail is in — so the critical
                            # startup transfers get the DMA engines to
                            # themselves. Per-queue FIFO order prioritizes
                            # by consumption time: the just-in-time wz/wh
                            # m1+ weight stream leads (on sync/gpsimd,
                            # keeping the scalar engine free for the
                            # activations), then x chunk 1 (needed ~25us
                            # later) and wo (needed ~50us later) trail so
                            # their bulk descriptors never starve the
                            # weight stream.
                            def gate(d):
                                add_dep_helper(
                                    d.ins, mm.ins, True, "bulk loads after start"
                                )

                            for mw in range(2, MH):
                                gate(nc.sync.dma_start(wz_s[:, mw], wz[:, mw]))
                                gate(nc.gpsimd.dma_start(wh_s[:, mw], wh[:, mw]))
                            if NCH > 1:
                                x_next = xpool.tile([P, KI, tc_len], BF16, tag="x")
                                gate(nc.gpsimd.dma_start(x_next[:], xT[:, 1]))
                            gate(nc.sync.dma_start(wo_s[:], wo[:]))
                    u_ps = zups.tile([P, tc_len], F32, tag="zu")
                    for k in range(KI):
                        nc.tensor.matmul(
                            u_ps[:],
                            wh_s[:, m, k, :],
                            x_s[:, k, :],
                            start=(k == 0),
                            stop=(k == KI - 1),
                        )
                    g_s = gpool.tile([P, tc_len], F32, tag="g")
                    # g = sigmoid(z + bz)
                    nc.scalar.activation(
                        g_s[:], z_ps[:], AF.Sigmoid, bias=bz_s[:, m : m + 1], scale=1.0
                    )
                    # a = 1 - g = sigmoid(-z - bz)
                    nc.scalar.activation(
                        a_s[:, m, :],
                        z_ps[:],
                        AF.Sigmoid,
                        bias=nbz_s[:, m : m + 1],
                        scale=-1.0,
                    )
                    # b = (u + bh) * g
                    nc.vector.scalar_tensor_tensor(
                        b_s[:, m, :], u_ps[:], bh_s[:, m : m + 1], g_s[:], AL.add, AL.mult
                    )
                    # h[:, t] = a[:, t] * h[:, t-1] + b[:, t]
                    init = 0.0 if c == 0 else h_prev[:, m, tc_len - 1 : tc_len]
                    nc.vector.tensor_tensor_scan(
                        h_s[:, m, :], a_s[:, m, :], b_s[:, m, :], init, AL.mult, AL.add
                    )

                # Output matmuls for the previous chunk, emitted after this
                # chunk's gate/update matmuls so the PE stream never has to
                # wait on the (serial) scan chain.
                if c > 0:
                    emit_out_chunk(c - 1, h_prev)
                h_prev = h_s
            emit_out_chunk(NCH - 1, h_prev, final=True)

    nc.compile()
    return nc


_CACHED_NC = None


def _get_nc():
    global _CACHED_NC
    if _CACHED_NC is None:
        _CACHED_NC = build_program()
    return _CACHED_NC


# Set by test harnesses that want a profile: kernel() stores the raw
# BassKernelResults of the last run here when TRACE is truthy.
TRACE = False
LAST_RESULTS = None


def _pack_weight(w):
    # [out_dim, in_dim] -> lhsT tiles [P, M_tiles, K_tiles, P] where
    # arr[p, m, k, q] = w[m*P + q, k*P + p]
    kd, md = w.shape[1] // P, w.shape[0] // P
    return np.ascontiguousarray(
        w.T.reshape(kd, P, md, P).transpose(1, 2, 0, 3).astype(NPBF16)
    )


def kernel(**inputs):
    global LAST_RESULTS
    xs = np.asarray(inputs["xs"], np.float32)
    Wz = np.asarray(inputs["Wz"], np.float32)
    bz = np.asarray(inputs["bz"], np.float32)
    Wh = np.asarray(inputs["Wh"], np.float32)
    bh = np.asarray(inputs["bh"], np.float32)
    Wo = np.asarray(inputs["Wo"], np.float32)
    bo = np.asarray(inputs["bo"], np.float32)

    KI, MH, MO, NCH = I // P, H // P, O // P, T // TC

    wz_t = _pack_weight(Wz)
    wh_t = _pack_weight(Wh)
    wo_t = _pack_weight(Wo)
    bias_p = np.concatenate(
        [
            bz.reshape(MH, P).T,
            (-bz).reshape(MH, P).T,
            bh.reshape(MH, P).T,
            bo.reshape(MO, P).T,
        ],
        axis=1,
    )
    bias_p = np.ascontiguousarray(bias_p, np.float32)

    in_maps = []
    for b in range(B):
        # [T, I] -> [P, NCH, KI, TC] with x[p, c, k, t] = xs[b, c*TC+t, k*P+p]
        xb = xs[b].astype(NPBF16).reshape(NCH, TC, KI, P)
        xb = np.ascontiguousarray(xb.transpose(3, 0, 2, 1))
        in_maps.append(
            {
                "xT": xb,
                "wz": wz_t,
                "wh": wh_t,
                "wo": wo_t,
                "biases": bias_p,
            }
        )

    nc = _get_nc()
    old_m = nc.m
    nc.m = get_hw_module(nc.m)
    try:
        res = run_bass_kernel_spmd(
            nc, in_maps, core_ids=list(range(B)), trace=bool(TRACE)
        )
    finally:
        nc.m = old_m
    LAST_RESULTS = res

    out_full = np.empty((B, T, O), np.float32)
    for b in range(B):
        # [P, MO, T] -> [O, T] -> [T, O]
        ob = res.results[b]["out"]
        out_full[b] = ob.transpose(1, 0, 2).reshape(O, T).T
    return out_full
